# revision 99
# baseline (speedup 1.0000x reference)
"""CapsuleLayer (dynamic routing, ROUTING_ITER=2) Bass/Tile kernel for TRN2.

Contract: kernel(x, weight) takes FULL inputs
  x:      [64, 2048, 1, 16] f32
  weight: [1, 2048, 32, 16, 16] f32
returns FULL output [64, 32, 16] f32.

Sharding: data-parallel over batch B=64 across 8 cores (8 per core),
weight replicated. Self-contained: hardcodes shapes, imports only
numpy/ml_dtypes/concourse.

Engine split: PE does u = W@x, all s-accumulations, and the V partition
broadcast; Act does the PSUM->SBUF u copies and exp; routing elementwise
runs in two lanes: DVE (14 chunks of 8 gh, bf16 2x mode) and Pool
(8 chunks of 2 gh), each with its own PSUM accumulator.
"""

from contextlib import ExitStack

import ml_dtypes
import numpy as np

import concourse.bacc as bacc
import concourse.bass as bass
import concourse.mybir as mybir
import concourse.tile as tile
from concourse.bass_utils import run_bass_kernel_spmd

F32 = mybir.dt.float32
BF16 = mybir.dt.bfloat16
AF = mybir.ActivationFunctionType
AX = mybir.AxisListType
ALU = mybir.AluOpType

EPS = 1e-8
J, D, E = 32, 16, 16
JD = J * D  # 512


def emit_capsule(tc, w2, xbd, d1, ds, out, db, xc, xm, n_in, b_loc=8, pch=2, npl=8):
    """Emit the per-core capsule program.

    DRAM tensors (APs):
      w2  [G, 8, E, JD] bf16  weight, host-permuted to [i, e, d, j], i=g*8+di
      xbd [128, 32*64]  bf16  block-diag x stationary for g<32 (head)
      xc  [128, (G-32)*8] bf16 compact x for g>=32, expanded on-chip
      d1  [128, 8] bf16       delta matrix * (1/32)  (s1 accumulation)
      ds  [128, 8] bf16       delta matrix * 1.0     (s2/s3 accumulation)
      db  [8, 128] f32        delta for V partition-broadcast matmul
      xm  [128, 8] bf16       block-diag expansion mask
      out [b_loc, JD] f32     squash(s3) output, (j,d) layout

    u layout in SBUF (bf16): partition p = (g%2)*64 + b*8 + di,
    free f = (g//2)*JD + d*J + j, where i = g*8 + di.
    """
    nc = tc.nc
    assert b_loc == 8
    G = n_in // 8
    GH = G // 2
    GDMA = 8   # groups per W DMA chunk
    CH = 8             # gh per DVE routing chunk
    PCH, NPL = pch, npl  # Pool-lane chunk size / count
    NDVE = (GH - NPL * PCH) // CH
    assert G % GDMA == 0 and NDVE * CH + NPL * PCH == GH

    ctx = ExitStack()
    singles = ctx.enter_context(tc.tile_pool(name="singles", bufs=1))
    small = ctx.enter_context(tc.tile_pool(name="small", bufs=1))

    u_sb = singles.tile([128, GH * JD], BF16)
    d1_sb = singles.tile([128, 8], BF16)
    ds_sb = singles.tile([128, 8], BF16)
    db_sb = singles.tile([8, 128], BF16)
    xm_sb = singles.tile([128, 8], BF16)
    nc.sync.dma_start(out=xm_sb, in_=xm)
    v_exp = singles.tile([128, JD], BF16)
    V = singles.tile([8, JD], F32)      # running sum of v vectors
    s_sb = singles.tile([8, JD], F32)
    eps8 = singles.tile([8, 1], F32)
    nc.vector.memset(eps8, EPS)

    # ---------- squash helpers (all on 8 partitions, tiny) ----------
    def squash_j(s_in, v_out):
        # v = squash(s, axis=j):  sq[b,d] = sum_j s^2;  v = s*sq/((1+sq)*sqrt(sq+eps))
        t2 = small.tile([8, JD], F32, tag="sqt2")
        nc.vector.tensor_mul(t2, s_in, s_in)
        sv = small.tile([8, 4, J], F32, tag="sqv")
        sq, a, t3, w = sv[:, 0, :D], sv[:, 1, :D], sv[:, 2, :D], sv[:, 3, :D]
        nc.vector.reduce_sum(out=sq, in_=t2.rearrange("p (d j) -> p d j", d=D), axis=AX.X)
        nc.scalar.activation(a, sq, AF.Sqrt, bias=eps8)
        nc.vector.scalar_tensor_tensor(t3, sq, 1.0, a, ALU.add, ALU.mult)
        nc.vector.reciprocal(w, t3)
        nc.vector.tensor_mul(w, w, sq)           # sq/((1+sq)a)
        wb = w.unsqueeze(2).broadcast_to([8, D, J])
        nc.vector.tensor_mul(v_out.rearrange("p (d j) -> p d j", d=D),
                             s_in.rearrange("p (d j) -> p d j", d=D), wb)

    def squash_d(s_in, v_out):
        # v = squash(s, axis=d): sq[b,j] = sum_d s^2
        t2 = small.tile([8, JD], F32, tag="sqt2")
        nc.vector.tensor_mul(t2, s_in, s_in)
        sv = small.tile([8, 4, J], F32, tag="sqv")
        sq, a, t3, w = sv[:, 0, :], sv[:, 1, :], sv[:, 2, :], sv[:, 3, :]
        nc.vector.reduce_sum(out=sq, in_=t2.rearrange("p (d j) -> p j d", d=D), axis=AX.X)
        nc.scalar.activation(a, sq, AF.Sqrt, bias=eps8)
        nc.vector.scalar_tensor_tensor(t3, sq, 1.0, a, ALU.add, ALU.mult)
        nc.vector.reciprocal(w, t3)
        nc.vector.tensor_mul(w, w, sq)
        wb = w.unsqueeze(1).broadcast_to([8, D, J])
        nc.vector.tensor_mul(v_out.rearrange("p (d j) -> p d j", d=D),
                             s_in.rearrange("p (d j) -> p d j", d=D), wb)

    vxp_pool = ctx.enter_context(tc.tile_pool(name="vxp", bufs=1, space="PSUM"))

    def refresh_v_exp():
        # replicate V[8,:] to all 128 partitions: PE matmul with delta
        # stationary, then one PSUM->SBUF cast copy on Act
        vb = small.tile([8, JD], BF16, tag="vb")
        nc.vector.tensor_copy(out=vb, in_=V)
        vxp = vxp_pool.tile([128, JD], F32, tag="vxp")
        nc.tensor.matmul(vxp, db_sb, vb, start=True, stop=True)
        nc.scalar.copy(out=v_exp, in_=vxp)

    # ---------- phase 1: W pass (u = W @ x), s1 accumulation ----------
    with tc.tile_pool(name="xsb", bufs=1) as xsb_pool, \
         tc.tile_pool(name="wp", bufs=3) as wp, \
         tc.tile_pool(name="up", bufs=3, space="PSUM") as up, \
         tc.tile_pool(name="sp", bufs=1, space="PSUM") as sp:
        xbd_sb = xsb_pool.tile([128, G * 64], BF16)
        xc_sb = xsb_pool.tile([128, (G - 32) * 8], BF16)
        # W chunk 0 first: the first matmul needs it AND the xbd head, and
        # it is the longer transfer. head (g 0..31) arrives block-diagonal
        # so PE can start immediately; the tail is shipped compact (8x fewer
        # bytes) and expanded on-chip by the otherwise-idle DVE
        wt0 = wp.tile([128, GDMA, JD], BF16, tag="wt")
        nc.sync.dma_start(
            out=wt0, in_=w2[0:GDMA].rearrange("g di e f -> (di e) g f"))
        nc.gpsimd.dma_start(out=xbd_sb[:, 0:32 * 64], in_=xbd)
        nc.gpsimd.dma_start(out=xc_sb, in_=xc)
        GT = G - 32
        HH = GT // 2
        for h in range(2):
            seg = xbd_sb[:, (32 + h * HH) * 64:(32 + (h + 1) * HH) * 64]
            segv = seg.rearrange("p (g b dd) -> p g b dd", b=8, dd=8)
            xcv = xc_sb[:, h * HH * 8:(h + 1) * HH * 8].rearrange(
                "p (g b) -> p g b", b=8)
            xcb = xcv.unsqueeze(3).broadcast_to([128, HH, 8, 8])
            xmb = xm_sb.unsqueeze(1).unsqueeze(1).broadcast_to([128, HH, 8, 8])
            nc.vector.tensor_mul(segv, xcb, xmb)
        s1_ps = sp.tile([8, JD], F32)
        for ci in range(G // GDMA):
            if ci == 0:
                wt = wt0
                nc.sync.dma_start(out=d1_sb, in_=d1)
                nc.sync.dma_start(out=ds_sb, in_=ds)
                nc.sync.dma_start(out=db_sb, in_=db)
            else:
                wt = wp.tile([128, GDMA, JD], BF16, tag="wt")
                # alternate issue queues so DMA issue overhead is not
                # serialized on one engine's DGE
                wsrc = w2[ci * GDMA:(ci + 1) * GDMA].rearrange(
                    "g di e f -> (di e) g f")
                if ci == G // GDMA - 1:
                    # split the final chunk so early groups' matmuls overlap
                    # the rest of the transfer (shorter tail)
                    q = GDMA // 4
                    for hq in range(4):
                        eng_q = nc.sync if hq % 2 == 0 else nc.gpsimd
                        eng_q.dma_start(out=wt[:, hq * q:(hq + 1) * q, :],
                                        in_=wsrc[:, hq * q:(hq + 1) * q, :])
                else:
                    (nc.sync if ci % 2 == 0 else nc.gpsimd).dma_start(
                        out=wt, in_=wsrc)
            for gq in range(GDMA // 4):
                pt = up.tile([128, 2 * JD], F32, tag="upt")
                for idx in range(4):
                    gl = gq * 4 + idx
                    g_abs = ci * GDMA + gl
                    nc.tensor.matmul(
                        pt[(gl % 2) * 64:(gl % 2) * 64 + 64,
                           (idx // 2) * JD:(idx // 2) * JD + JD],
                        xbd_sb[:, g_abs * 64:(g_abs + 1) * 64],
                        wt[:, gl, :], start=True, stop=True)
                gh0 = ci * (GDMA // 2) + gq * 2
                nc.scalar.copy(out=u_sb[:, gh0 * JD:(gh0 + 2) * JD], in_=pt)
                for gh in (gh0, gh0 + 1):
                    nc.tensor.matmul(s1_ps, d1_sb, u_sb[:, gh * JD:(gh + 1) * JD],
                                     start=(gh == 0), stop=(gh == GH - 1))
        nc.vector.tensor_copy(out=s_sb, in_=s1_ps)

    squash_j(s_sb, V)      # V = v1
    refresh_v_exp()

    # ---------- routing pass (T = u.V, softmax, s = sum_i c*u) ----------
    # Two decoupled lanes: DVE owns gh 0..103 (13 chunks of 8), Pool owns
    # gh 104..127 (4 chunks of 6). Each lane accumulates into its own PSUM
    # tile so a slow lane never stalls the other lane's PE chain. Pool-lane
    # work is emitted in stages interleaved with DVE chunks so every engine
    # FIFO receives work roughly in readiness order.
    def routing_pass(final):
        # prod is big (8KB) and consumed late by PE -> own 3-deep pool; the
        # tree scratch is chunk-local -> 2-deep; the Act-coupled and softmax
        # tiles (tt/eT/c/se) are tiny but chain Act to DVE via WAR when
        # under-buffered -> 4-deep.
        with tc.tile_pool(name="rp3", bufs=4) as rp3, \
             tc.tile_pool(name="rv2", bufs=2) as rv2, \
             tc.tile_pool(name="rv4", bufs=4) as rv4, \
             tc.tile_pool(name="rgp", bufs=2) as rgp, \
             tc.tile_pool(name="rg2", bufs=2) as rg2, \
             tc.tile_pool(name="rg4", bufs=4) as rg4, \
             tc.tile_pool(name="spp", bufs=1, space="PSUM") as spp:
            s_ps = spp.tile([8, JD], F32)
            if NPL:
                s_ps2 = spp.tile([8, JD], F32)
            else:
                s_ps2 = None

            def chunk_tiles(prod_pool, mid_pool, sm_pool, n):
                prod = prod_pool.tile([128, n, JD], BF16, tag="prod")
                t1 = mid_pool.tile([128, n, 8, J], BF16, tag="t1")
                t2 = mid_pool.tile([128, n, 4, J], BF16, tag="t2")
                t3 = mid_pool.tile([128, n, 2, J], BF16, tag="t3")
                tt = sm_pool.tile([128, n, J], BF16, tag="tt")
                eT = sm_pool.tile([128, n, J], BF16, tag="eT")
                c = sm_pool.tile([128, n, J], BF16, tag="c")
                return prod, t1, t2, t3, tt, eT, c

            def logits(eng, gh0, n, prod, t1, t2, t3, tt, eT):
                fs = gh0 * JD
                u_ch = u_sb[:, fs:fs + n * JD].rearrange("p (g f) -> p g f", g=n)
                vb = v_exp.unsqueeze(1).broadcast_to([128, n, JD])
                eng.tensor_mul(prod, u_ch, vb)
                p4 = prod.rearrange("p g (d j) -> p g d j", d=D)
                eng.tensor_add(t1, p4[:, :, 0:8, :], p4[:, :, 8:16, :])
                eng.tensor_add(t2, t1[:, :, 0:4, :], t1[:, :, 4:8, :])
                eng.tensor_add(t3, t2[:, :, 0:2, :], t2[:, :, 2:4, :])
                eng.tensor_add(tt.unsqueeze(2),
                               t3[:, :, 0:1, :], t3[:, :, 1:2, :])

            def y_of(eng, gh0, n, prod, c):
                fs = gh0 * JD
                u_ch = u_sb[:, fs:fs + n * JD].rearrange("p (g f) -> p g f", g=n)
                u4 = u_ch.rearrange("p g (d j) -> p g d j", d=D)
                p4 = prod.rearrange("p g (d j) -> p g d j", d=D)
                cb = c.unsqueeze(2).broadcast_to([128, n, D, J])
                eng.tensor_mul(p4, u4, cb)  # y over prod (dead after t1)

            # ---- DVE-lane chunk stages ----
            dve_state = {}

            def dve_stage1(k):
                gh0 = k * CH
                tiles = chunk_tiles(rp3, rv2, rv4, CH)
                prod, t1, t2, t3, tt, eT, c = tiles
                logits(nc.vector, gh0, CH, prod, t1, t2, t3, tt, eT)
                nc.scalar.activation(eT, tt, AF.Exp)
                se = rv4.tile([128, CH], F32, tag="se")
                r = rv4.tile([128, CH], F32, tag="r")
                dve_state[k] = (tiles, se, r)

            def dve_stage2(k):
                (prod, t1, t2, t3, tt, eT, c), se, r = dve_state.pop(k)
                gh0 = k * CH
                nc.vector.reduce_sum(out=se, in_=eT, axis=AX.X)
                nc.vector.reciprocal(r, se)
                rb = r.unsqueeze(2).broadcast_to([128, CH, J])
                nc.vector.tensor_mul(c, eT, rb)
                y_of(nc.vector, gh0, CH, prod, c)
                for q in range(CH):
                    gh = gh0 + q
                    nc.tensor.matmul(s_ps, ds_sb, prod[:, q, :],
                                     start=(gh == 0), stop=(gh == CH * NDVE - 1))

            # ---- Pool-lane chunk stages ----
            pl_state = {}

            def pool_stageA(i):
                gh0 = NDVE * CH + i * PCH
                tiles = chunk_tiles(rgp, rg2, rg4, PCH)
                prod, t1, t2, t3, tt, eT, c = tiles
                logits(nc.gpsimd, gh0, PCH, prod, t1, t2, t3, tt, eT)
                pl_state[i] = tiles

            def pool_stageB1(i):
                prod, t1, t2, t3, tt, eT, c = pl_state[i]
                nc.scalar.activation(eT, tt, AF.Exp)
                # softmax denominator via j-tree on Pool (disjoint slices)
                sj = rg4.tile([128, PCH, 32], BF16, tag="sj")
                nc.gpsimd.tensor_add(sj[:, :, 0:16], eT[:, :, 0:16], eT[:, :, 16:32])
                nc.gpsimd.tensor_add(sj[:, :, 16:24], sj[:, :, 0:8], sj[:, :, 8:16])
                nc.gpsimd.tensor_add(sj[:, :, 24:28], sj[:, :, 16:20], sj[:, :, 20:24])
                nc.gpsimd.tensor_add(sj[:, :, 28:30], sj[:, :, 24:26], sj[:, :, 26:28])
                nc.gpsimd.tensor_add(sj[:, :, 30:31], sj[:, :, 28:29], sj[:, :, 29:30])
                pl_state[i] = (pl_state[i], sj)

            def pool_stageB2(i):
                # 1/se on DVE, emitted late enough that DVE never waits on
                # the Pool lane when it reaches this FIFO slot
                (prod, t1, t2, t3, tt, eT, c), sj = pl_state[i]
                gh0 = NDVE * CH + i * PCH
                rp = rg4.tile([128, PCH], F32, tag="rp")
                nc.vector.reciprocal(rp, sj[:, :, 30])
                rpb = rp.unsqueeze(2).broadcast_to([128, PCH, J])
                nc.gpsimd.tensor_mul(c, eT, rpb)
                y_of(nc.gpsimd, gh0, PCH, prod, c)
                pl_state[i] = (prod,)

            def pool_mm(i):
                prod = pl_state.pop(i)[0]
                gh0 = NDVE * CH + i * PCH
                for q in range(PCH):
                    gh = gh0 + q
                    nc.tensor.matmul(s_ps2, ds_sb, prod[:, q, :],
                                     start=(gh == CH * NDVE),
                                     stop=(gh == GH - 1))

            # ---- interleaved emission ----
            def slot(i):
                return (i * NDVE) // NPL
            A_AT, B1_AT, B2_AT, M_AT = {}, {}, {}, {}
            for i in range(NPL):
                A_AT.setdefault(min(slot(i), NDVE - 1), []).append(i)
                B1_AT.setdefault(slot(i) + 2, []).append(i)
                B2_AT.setdefault(slot(i) + 3, []).append(i)
                M_AT.setdefault(slot(i) + 4, []).append(i)
            for k in range(NDVE):
                for i in A_AT.get(k, []):
                    pool_stageA(i)
                dve_stage1(k)
                if k > 0:
                    dve_stage2(k - 1)
                for i in B1_AT.get(k, []):
                    pool_stageB1(i)
                for i in B2_AT.get(k, []):
                    pool_stageB2(i)
                for i in M_AT.get(k, []):
                    pool_mm(i)
            dve_stage2(NDVE - 1)
            for i in range(NPL):
                if slot(i) + 2 > NDVE - 1:
                    pool_stageB1(i)
                if slot(i) + 3 > NDVE - 1:
                    pool_stageB2(i)
                if slot(i) + 4 > NDVE - 1:
                    pool_mm(i)
            if NPL:
                s2sb = small.tile([8, JD], F32, tag="s2sb")
                nc.scalar.copy(out=s2sb, in_=s_ps2)
                nc.vector.tensor_add(s_sb, s_ps, s2sb)
            else:
                nc.vector.tensor_copy(out=s_sb, in_=s_ps)
        if not final:
            v2 = small.tile([8, JD], F32, tag="v2")
            squash_j(s_sb, v2)
            nc.vector.tensor_add(V, V, v2)
            refresh_v_exp()
        else:
            vout = small.tile([8, JD], F32, tag="vout")
            squash_d(s_sb, vout)
            nc.sync.dma_start(out=out, in_=vout)

    routing_pass(final=False)   # iteration 2 (uses V=v1)
    routing_pass(final=True)    # final (uses V=v1+v2)
    ctx.close()


def build_module(n_in=2048, b_loc=8, num_devices=8, enable_asserts=False, pch=2, npl=8):
    nc = bacc.Bacc("TRN2", target_bir_lowering=False, debug=False,
                   num_devices=num_devices, enable_asserts=enable_asserts)
    G = n_in // 8
    w2 = nc.dram_tensor("w2", [G, 8, E, JD], BF16, kind="ExternalInput").ap()
    db = nc.dram_tensor("db", [8, 128], BF16, kind="ExternalInput").ap()
    xm = nc.dram_tensor("xm", [128, 8], BF16, kind="ExternalInput").ap()
    xbd = nc.dram_tensor("xbd", [128, 32 * 64], BF16, kind="ExternalInput").ap()
    xc = nc.dram_tensor("xc", [128, (G - 32) * 8], BF16, kind="ExternalInput").ap()
    d1 = nc.dram_tensor("d1", [128, 8], BF16, kind="ExternalInput").ap()
    ds = nc.dram_tensor("ds", [128, 8], BF16, kind="ExternalInput").ap()
    out = nc.dram_tensor("out", [b_loc, JD], F32, kind="ExternalOutput").ap()
    with tile.TileContext(nc) as tc:
        emit_capsule(tc, w2, xbd, d1, ds, out, db=db, xc=xc, xm=xm, n_in=n_in, b_loc=b_loc, pch=pch, npl=npl)
    nc.compile()
    return nc


def host_prep_w(weight, n_in):
    # weight [1, N, J, D, E] -> w2 [G, 8, E, J*D] with free layout (d, j)
    w2 = np.ascontiguousarray(weight[0].transpose(0, 3, 2, 1))  # [N, E, D, J]
    return w2.reshape(n_in // 8, 8, E, JD).astype(ml_dtypes.bfloat16)


def host_prep_xbd(xs, n_in):
    # xs [b_loc, N, E] -> (xbd_head [128, 32*64] block-diag for g<32,
    #                      xc_tail [128, (G-32)*8] compact for g>=32)
    G = n_in // 8
    t = xs.reshape(8, G, 8, E).transpose(2, 3, 1, 0)  # [di, e, G, b]
    head = np.zeros((8, E, 32, 8, 8), np.float32)     # [di, e, g, b, di']
    for di in range(8):
        head[di, :, :, :, di] = t[di, :, 0:32, :]
    xbd_head = head.reshape(128, 32 * 64).astype(ml_dtypes.bfloat16)
    xc_tail = np.ascontiguousarray(t[:, :, 32:, :]).reshape(
        128, (G - 32) * 8).astype(ml_dtypes.bfloat16)
    return xbd_head, xc_tail


def host_prep_deltas():
    p = np.arange(128)
    bofp = (p // 8) % 8
    d1 = np.zeros((128, 8), np.float32)
    ds = np.zeros((128, 8), np.float32)
    d1[p, bofp] = 1.0 / 32.0
    ds[p, bofp] = 1.0
    db = np.zeros((8, 128), np.float32)
    db[bofp, p] = 1.0
    xm = np.zeros((128, 8), np.float32)
    xm[p, p // 16] = 1.0
    return (d1.astype(ml_dtypes.bfloat16), ds.astype(ml_dtypes.bfloat16),
            db.astype(ml_dtypes.bfloat16), xm.astype(ml_dtypes.bfloat16))


_CACHE = {}
LAST_EXEC_NS = None


def kernel(x, weight, trace=False):
    B, N_in = 64, 2048
    n_cores = 8
    b_loc = B // n_cores
    key = (N_in, b_loc, n_cores)
    if key not in _CACHE:
        _CACHE[key] = build_module(n_in=N_in, b_loc=b_loc, num_devices=n_cores)
    nc = _CACHE[key]

    x = np.asarray(x, dtype=np.float32)
    weight = np.asarray(weight, dtype=np.float32)
    w2 = host_prep_w(weight, N_in)
    d1, ds, db, xm = host_prep_deltas()
    in_maps = []
    for c in range(n_cores):
        xs = np.ascontiguousarray(x[c * b_loc:(c + 1) * b_loc, :, 0, :])
        xbd_head, xc_tail = host_prep_xbd(xs, N_in)
        in_maps.append({
            "w2": w2,
            "xbd": xbd_head,
            "xc": xc_tail,
            "d1": d1,
            "ds": ds,
            "db": db,
            "xm": xm,
        })
    global LAST_EXEC_NS
    res = run_bass_kernel_spmd(nc, in_maps, core_ids=list(range(n_cores)),
                               trace=trace)
    LAST_EXEC_NS = res.exec_time_ns
    outs = [r["out"].reshape(b_loc, D, J).transpose(0, 2, 1) for r in res.results]
    return np.ascontiguousarray(np.concatenate(outs, axis=0))
